# revision 24
# baseline (speedup 1.0000x reference)
"""Trainium2 Bass kernel for nn_MixedGatedMLP (4-bit quantized gated MLP + LoRA).

Strategy: tensor-parallel over d_ff across 8 NeuronCores (F padded 11008->11264,
1408 rows/core).  V2 pipeline: a free-running dequant SWEEP (DVE is_equal
masked-sum + ACT u8->bf16 converts) writes bf16 weights to per-window DRAM
staging tensors; the matmul stages consume them so TensorE never shares an
engine queue with the sweep.  Stage B (gate/up GEMMs + silu-gating, gating
mult on GPSIMD) produces x3; stage C (down GEMM, one pass per 1024-wide
d-quarter) feeds a ReduceScatter that sums the 8 partial y3 and scatters
tokens, so core i directly receives its 512 output tokens.
"""

import sys

for _p in ("/opt/trn_rl_repo", "/root/.axon_site/_ro/trn_rl_repo"):
    if _p not in sys.path:
        sys.path.append(_p)

from contextlib import ExitStack

import numpy as np
import ml_dtypes

import concourse.bass as bass
import concourse.mybir as mybir
import concourse.tile as tile
from concourse import bacc
from concourse.bass_utils import run_bass_kernel_spmd

BF16 = ml_dtypes.bfloat16
NCORES = 8
ALU = mybir.AluOpType
AFT = mybir.ActivationFunctionType


class Cfg:
    def __init__(self, D=4096, T=4096, F=11008, R=16, block=64, ncores=8):
        self.D = D              # d_model
        self.T = T              # tokens
        self.F = F              # true d_ff
        self.R = R              # lora rank
        self.block = block      # absmax block size
        self.ncores = ncores
        unit = 2 * block * ncores
        self.FP = ((F + unit - 1) // unit) * unit     # 11264
        self.FS = self.FP // ncores                   # 1408 f rows/core
        self.TS = T // ncores                         # 512 out tokens/core
        self.DP = D // 256                            # 16 byte-row chunks
        self.NT = T // 512                            # 8 token tiles
        # gate/up f-windows (narrow first => short PE prologue)
        self.fwins = [(0, 256), (256, 512), (768, 640)]
        assert sum(w for _, w in self.fwins) == self.FS
        # down d-quarters
        self.DDQ = 1024
        self.n_q = D // self.DDQ                      # 4
        self.NFG = self.FS // 128                     # 11 f-groups
        # down byte-pair chunks (pairs of f): [(j0, j1), ...] <=128 each
        self.j_chunks = []
        j0 = 0
        npairs = self.FS // 2
        while j0 < npairs:
            j1 = min(j0 + 128, npairs)
            self.j_chunks.append((j0, j1))
            j0 = j1
        self.use_silu = True


def _dperm(D):
    """Row order of xT: per 256-d chunk, evens then odds."""
    idx = []
    for c in range(D // 256):
        base = 256 * c
        idx.extend(range(base, base + 256, 2))
        idx.extend(range(base + 1, base + 256, 2))
    return np.array(idx)


def _fperm_local(cfg):
    """Within-shard f order: per down j-chunk, even f (2j) then odd f (2j+1)."""
    idx = []
    for (j0, j1) in cfg.j_chunks:
        idx.extend(2 * j for j in range(j0, j1))
        idx.extend(2 * j + 1 for j in range(j0, j1))
    return np.array(idx)


def build_graph(cfg: Cfg):
    nc = bacc.Bacc(None, num_devices=cfg.ncores)
    dt = mybir.dt
    D, T, FS, R = cfg.D, cfg.T, cfg.FS, cfg.R

    # ---- external inputs (per-core) ----
    xT = nc.dram_tensor("xT", [D, T], dt.bfloat16, kind="ExternalInput")
    g_bytes = nc.dram_tensor("g_bytes", [D // 2, FS], dt.uint8, kind="ExternalInput")
    u_bytes = nc.dram_tensor("u_bytes", [D // 2, FS], dt.uint8, kind="ExternalInput")
    d_bytes = nc.dram_tensor("d_bytes", [FS // 2, D], dt.uint8, kind="ExternalInput")
    g_am = nc.dram_tensor("g_am", [D // 2, FS], dt.bfloat16, kind="ExternalInput")
    u_am = nc.dram_tensor("u_am", [D // 2, FS], dt.bfloat16, kind="ExternalInput")
    d_am = nc.dram_tensor("d_am", [FS // 2, D], dt.bfloat16, kind="ExternalInput")
    code_rep = nc.dram_tensor("code_rep", [128, 16], dt.float32, kind="ExternalInput")
    a_gu = nc.dram_tensor("a_gu", [D, 2 * R], dt.bfloat16, kind="ExternalInput")
    b_g = nc.dram_tensor("b_g", [R, FS], dt.bfloat16, kind="ExternalInput")
    b_u = nc.dram_tensor("b_u", [R, FS], dt.bfloat16, kind="ExternalInput")
    a_d = nc.dram_tensor("a_d", [FS, R], dt.bfloat16, kind="ExternalInput")
    b_d = nc.dram_tensor("b_d", [R, D], dt.bfloat16, kind="ExternalInput")

    y_q = [nc.dram_tensor(f"y_q{q}", [cfg.TS, cfg.DDQ], dt.bfloat16,
                          kind="ExternalOutput")
           for q in range(cfg.n_q)]

    # ---- internal DRAM staging ----
    wg_w = [nc.dram_tensor(f"wg_w{i}", [D, fw], dt.bfloat16, kind="Internal")
            for i, (_, fw) in enumerate(cfg.fwins)]
    wu_w = [nc.dram_tensor(f"wu_w{i}", [D, fw], dt.bfloat16, kind="Internal")
            for i, (_, fw) in enumerate(cfg.fwins)]
    wd_q = [nc.dram_tensor(f"wd_q{q}", [FS, cfg.DDQ], dt.bfloat16, kind="Internal")
            for q in range(cfg.n_q)]
    x3_dram = nc.dram_tensor("x3_dram", [FS, T], dt.bfloat16, kind="Internal")
    rs_in = [nc.dram_tensor(f"rs_in{q}", [T, cfg.DDQ], dt.bfloat16, kind="Internal")
             for q in range(cfg.n_q)]
    a2a_out = [nc.dram_tensor(f"a2a_out{q}", [T, cfg.DDQ], dt.bfloat16,
                              kind="Internal")
               for q in range(cfg.n_q)]

    rg = [list(range(cfg.ncores))]

    with tile.TileContext(nc) as tc, ExitStack() as ctx:
        const_pool = ctx.enter_context(tc.tile_pool(name="const", bufs=1))
        code_sb = const_pool.tile([128, 16], dt.float32)
        nc.sync.dma_start(code_sb[:], code_rep[:])
        agu_sb = const_pool.tile([128, D // 128, 2 * R], dt.bfloat16)
        nc.sync.dma_start(agu_sb[:], a_gu.rearrange("(c p) r -> p c r", p=128))
        bg_sb = const_pool.tile([R, FS], dt.bfloat16)
        nc.sync.dma_start(bg_sb[:], b_g[:])
        bu_sb = const_pool.tile([R, FS], dt.bfloat16)
        nc.sync.dma_start(bu_sb[:], b_u[:])
        ad_sb = const_pool.tile([128, FS // 128, R], dt.bfloat16)
        nc.sync.dma_start(ad_sb[:], a_d.rearrange("(c p) r -> p c r", p=128))
        bd_sb = const_pool.tile([R, D], dt.bfloat16)
        nc.sync.dma_start(bd_sb[:], b_d[:])
        # persistent lora activations (R=16 partitions)
        xag_sb = const_pool.tile([R, T], dt.bfloat16)
        xau_sb = const_pool.tile([R, T], dt.bfloat16)
        x3a_sb = const_pool.tile([R, T], dt.bfloat16)

        # ============ dequant sweep (DVE + ACT) ============
        # These pools stay open for the whole kernel: if their SBUF zones
        # were released and reused by the matmul-stage pools, the reuse
        # dependency would serialize the matmuls behind the entire sweep.
        dqs = ctx.enter_context(tc.tile_pool(name="dqs", bufs=2))
        wst = ctx.enter_context(tc.tile_pool(name="wst", bufs=2))
        if True:
            def dq_chain(B, S, fw2, extract, W):
                """Masked-sum codebook lookup on byte tile B [128, fw2]:
                extract nibble per `extract` ('hi' | 'lo' | 'stacked' using
                the per-partition shift vector), then W = code[n] * S."""
                U = dqs.tile([128, fw2], dt.uint8, tag="u")
                if extract == "hi":
                    nc.vector.tensor_scalar(U[:], B[:], 4, None,
                                            ALU.logical_shift_right)
                elif extract == "lo":
                    nc.vector.tensor_scalar(U[:], B[:], 15, None,
                                            ALU.bitwise_and)
                else:  # stacked: hi on partitions [0,64), lo on [64,128)
                    nc.vector.tensor_scalar(U[0:64, :], B[0:64, :], 4, None,
                                            ALU.logical_shift_right)
                    nc.vector.tensor_scalar(U[64:128, :], B[64:128, :], 15,
                                            None, ALU.bitwise_and)
                X = dqs.tile([128, fw2], dt.bfloat16, tag="x")
                nc.scalar.copy(X[:], U[:])
                acc0 = dqs.tile([128, fw2], dt.bfloat16, tag="a0")
                acc1 = dqs.tile([128, fw2], dt.bfloat16, tag="a1")
                tk = dqs.tile([128, fw2], dt.bfloat16, tag="tk")
                nc.vector.tensor_scalar(acc0[:], X[:], 0.0, code_sb[:, 0:1],
                                        ALU.is_equal, ALU.mult)
                nc.vector.tensor_scalar(acc1[:], X[:], 1.0, code_sb[:, 1:2],
                                        ALU.is_equal, ALU.mult)
                for k in range(2, 16):
                    acc = acc0 if (k % 2 == 0) else acc1
                    nc.vector.tensor_scalar(tk[:], X[:], float(k),
                                            code_sb[:, k:k + 1],
                                            ALU.is_equal, ALU.mult)
                    nc.vector.tensor_tensor(acc[:], acc[:], tk[:], ALU.add)
                nc.vector.tensor_tensor(acc0[:], acc0[:], acc1[:], ALU.add)
                nc.vector.tensor_tensor(W[:], acc0[:], S[:], ALU.mult)

            # gate/up, window-major so stage B unblocks per window; gate and
            # up share each chain as a [g|u] double-width superplane to halve
            # per-instruction bubble overhead
            for wi, (f0, fw) in enumerate(cfg.fwins):
                for c in range(cfg.DP):
                    rsl = slice(128 * c, 128 * (c + 1))
                    B2 = dqs.tile([128, 2 * fw], dt.uint8, tag="bq")
                    S2 = dqs.tile([128, 2 * fw], dt.bfloat16, tag="sq")
                    nc.sync.dma_start(B2[:, 0:fw], g_bytes[rsl, f0:f0 + fw])
                    nc.sync.dma_start(B2[:, fw:], u_bytes[rsl, f0:f0 + fw])
                    nc.sync.dma_start(S2[:, 0:fw], g_am[rsl, f0:f0 + fw])
                    nc.sync.dma_start(S2[:, fw:], u_am[rsl, f0:f0 + fw])
                    for ni, ex in ((0, "hi"), (1, "lo")):
                        W2 = wst.tile([128, 2 * fw], dt.bfloat16, tag="wsh")
                        dq_chain(B2, S2, 2 * fw, ex, W2)
                        osl = slice(256 * c + 128 * ni,
                                    256 * c + 128 * (ni + 1))
                        nc.sync.dma_start(wg_w[wi][osl, :], W2[:, 0:fw])
                        nc.sync.dma_start(wu_w[wi][osl, :], W2[:, fw:])
            # down, quarter-major
            for q in range(cfg.n_q):
                dd = slice(cfg.DDQ * q, cfg.DDQ * (q + 1))
                for ic, (j0, j1) in enumerate(cfg.j_chunks):
                    pc = j1 - j0
                    B2 = dqs.tile([128, cfg.DDQ], dt.uint8, tag="bq")
                    S2 = dqs.tile([128, cfg.DDQ], dt.bfloat16, tag="sq")
                    if pc == 128:
                        nc.sync.dma_start(B2[:], d_bytes[j0:j1, dd])
                        nc.sync.dma_start(S2[:], d_am[j0:j1, dd])
                        for ni, ex in ((0, "hi"), (1, "lo")):
                            W2 = wst.tile([128, cfg.DDQ], dt.bfloat16,
                                          tag="wsh")
                            dq_chain(B2, S2, cfg.DDQ, ex, W2)
                            osl = slice(256 * ic + 128 * ni,
                                        256 * ic + 128 * (ni + 1))
                            nc.sync.dma_start(wd_q[q][osl, :], W2[:])
                    else:
                        # ragged tail (64 pairs): duplicate the byte rows on
                        # both partition halves, extract hi on [0,64) and lo
                        # on [64,128) via the per-partition shift vector
                        nc.sync.dma_start(B2[0:pc, :], d_bytes[j0:j1, dd])
                        nc.sync.dma_start(B2[pc:2 * pc, :], d_bytes[j0:j1, dd])
                        nc.sync.dma_start(S2[0:pc, :], d_am[j0:j1, dd])
                        nc.sync.dma_start(S2[pc:2 * pc, :], d_am[j0:j1, dd])
                        W2 = wst.tile([128, cfg.DDQ], dt.bfloat16, tag="wsh")
                        dq_chain(B2, S2, cfg.DDQ, "stacked", W2)
                        nc.sync.dma_start(
                            wd_q[q][256 * ic:256 * ic + 2 * pc, :],
                            W2[0:2 * pc, :])

        # ================= stage B: gate/up matmuls -> x3 =================
        with (
            tc.tile_pool(name="w", bufs=34) as w_pool,
            tc.tile_pool(name="xt", bufs=34) as xt_pool,
            tc.tile_pool(name="p1", bufs=3) as p1_pool,
            tc.tile_pool(name="ps1", bufs=3, space="PSUM") as psum1,
            tc.tile_pool(name="psa", bufs=2, space="PSUM") as psuma,
        ):
            # lora prologue: x@[Ag|Au] for all token tiles (only needs x)
            for t in range(cfg.NT):
                tt = slice(512 * t, 512 * (t + 1))
                xts = []
                for ci in range(2 * cfg.DP):
                    xt_t = xt_pool.tile([128, 512], dt.bfloat16, tag="xt")
                    nc.sync.dma_start(xt_t[:], xT[128 * ci:128 * (ci + 1), tt])
                    xts.append(xt_t)
                for ri, dst in ((0, xag_sb), (1, xau_sb)):
                    pa2 = psuma.tile([R, 512], dt.float32, tag="pa2")
                    for ci in range(2 * cfg.DP):
                        nc.tensor.matmul(pa2[:],
                                         agu_sb[:, ci, R * ri:R * (ri + 1)],
                                         xts[ci][:], start=(ci == 0),
                                         stop=(ci == 2 * cfg.DP - 1))
                    nc.scalar.copy(dst[:, tt], pa2[:])

            for wi, (f0, fw) in enumerate(cfg.fwins):
                wg = []
                wu = []
                for ci in range(2 * cfg.DP):
                    tg_ = w_pool.tile([128, fw], dt.bfloat16, tag="wg")
                    nc.sync.dma_start(
                        tg_[:], wg_w[wi][128 * ci:128 * (ci + 1), :])
                    wg.append(tg_)
                    tu_ = w_pool.tile([128, fw], dt.bfloat16, tag="wu")
                    nc.sync.dma_start(
                        tu_[:], wu_w[wi][128 * ci:128 * (ci + 1), :])
                    wu.append(tu_)
                for t in range(cfg.NT):
                    tt = slice(512 * t, 512 * (t + 1))
                    xts = []
                    for ci in range(2 * cfg.DP):
                        xt_t = xt_pool.tile([128, 512], dt.bfloat16, tag="xt")
                        nc.sync.dma_start(
                            xt_t[:], xT[128 * ci:128 * (ci + 1), tt])
                        xts.append(xt_t)
                    for g in range(fw // 128):
                        fg = slice(128 * g, 128 * (g + 1))
                        fga = slice(f0 + 128 * g, f0 + 128 * (g + 1))
                        pg = psum1.tile([128, 512], dt.float32, tag="pg")
                        pu = psum1.tile([128, 512], dt.float32, tag="pu")
                        for ci in range(2 * cfg.DP):
                            nc.tensor.matmul(pg[:], wg[ci][:, fg], xts[ci][:],
                                             start=(ci == 0), stop=False)
                        nc.tensor.matmul(pg[:], bg_sb[:, fga], xag_sb[:, tt],
                                         start=False, stop=True)
                        for ci in range(2 * cfg.DP):
                            nc.tensor.matmul(pu[:], wu[ci][:, fg], xts[ci][:],
                                             start=(ci == 0), stop=False)
                        nc.tensor.matmul(pu[:], bu_sb[:, fga], xau_sb[:, tt],
                                         start=False, stop=True)
                        # silu on ACT; gating mult on GPSIMD (DVE is busy
                        # with the dequant sweep; GPSIMD has no PSUM access,
                        # so ACT also stages pu into SBUF)
                        sg = p1_pool.tile([128, 512], dt.bfloat16, tag="sg")
                        if cfg.use_silu:
                            nc.scalar.activation(sg[:], pg[:], AFT.Silu)
                        else:
                            nc.scalar.activation(sg[:], pg[:], AFT.Sigmoid)
                            nc.gpsimd.tensor_tensor(sg[:], sg[:], pg[:],
                                                    ALU.mult)
                        pu_s = p1_pool.tile([128, 512], dt.bfloat16, tag="pus")
                        nc.scalar.copy(pu_s[:], pu[:])
                        x3t = p1_pool.tile([128, 512], dt.bfloat16, tag="x3t")
                        nc.gpsimd.tensor_tensor(x3t[:], sg[:], pu_s[:],
                                                ALU.mult)
                        nc.sync.dma_start(x3_dram[fga, tt], x3t[:])

        # ================= stage C: down matmuls + AllToAll+reduce ========
        with (
            tc.tile_pool(name="wd", bufs=24) as wd_pool,
            tc.tile_pool(name="x3q", bufs=24) as x3q_pool,
            tc.tile_pool(name="yb", bufs=6) as yb_pool,
            tc.tile_pool(name="red", bufs=cfg.ncores + 2) as red_pool,
            tc.tile_pool(name="ps2", bufs=3, space="PSUM") as psum2,
            tc.tile_pool(name="psb", bufs=2, space="PSUM") as psumb,
        ):
            def reduce_q(q):
                """Sum the 8 exchanged partials for this core's tokens (DVE;
                idle once the sweep is done)."""
                for ts in range(cfg.TS // 128):
                    for dj in range(2):
                        dsl = slice(512 * dj, 512 * (dj + 1))
                        parts = []
                        for j in range(cfg.ncores):
                            pt = red_pool.tile([128, 512], dt.bfloat16,
                                               tag="rp")
                            r0 = cfg.TS * j + 128 * ts
                            nc.sync.dma_start(
                                pt[:], a2a_out[q][r0:r0 + 128, dsl])
                            parts.append(pt)
                        for lvl in (4, 2):
                            for j in range(lvl):
                                nc.vector.tensor_tensor(
                                    parts[j][:], parts[j][:],
                                    parts[j + lvl][:], ALU.add)
                        yf = red_pool.tile([128, 512], dt.bfloat16, tag="yf")
                        nc.vector.tensor_tensor(yf[:], parts[0][:],
                                                parts[1][:], ALU.add)
                        nc.sync.dma_start(
                            y_q[q][128 * ts:128 * (ts + 1), dsl], yf[:])

            for q in range(cfg.n_q):
                dds = slice(cfg.DDQ * q, cfg.DDQ * (q + 1))
                wd = []
                for g in range(cfg.NFG):
                    wt = wd_pool.tile([128, cfg.DDQ], dt.bfloat16, tag="wd")
                    nc.sync.dma_start(
                        wt[:], wd_q[q][128 * g:128 * (g + 1), :])
                    wd.append(wt)
                for tq in range(cfg.NT):  # 512-token quads
                    tqs = slice(512 * tq, 512 * (tq + 1))
                    x3q = []
                    for g in range(cfg.NFG):
                        xq = x3q_pool.tile([128, 512], dt.bfloat16, tag="x3q")
                        nc.sync.dma_start(
                            xq[:], x3_dram[128 * g:128 * (g + 1), tqs])
                        x3q.append(xq)
                    if q == 0:
                        # x3^T @ A_d for these tokens (needed by all lora
                        # stops); do all 4 subtiles first so the ACT copies
                        # pipeline ahead of the stop matmuls
                        for ts in range(4):
                            tcs = slice(128 * ts, 128 * (ts + 1))
                            tca = slice(512 * tq + 128 * ts,
                                        512 * tq + 128 * (ts + 1))
                            pa = psumb.tile([R, 128], dt.float32, tag="pa")
                            for g in range(cfg.NFG):
                                nc.tensor.matmul(pa[:], ad_sb[:, g, :],
                                                 x3q[g][:, tcs],
                                                 start=(g == 0),
                                                 stop=(g == cfg.NFG - 1))
                            nc.scalar.copy(x3a_sb[:, tca], pa[:])
                    for ts in range(4):
                        tcs = slice(128 * ts, 128 * (ts + 1))
                        tca = slice(512 * tq + 128 * ts,
                                    512 * tq + 128 * (ts + 1))
                        pds = [psum2.tile([128, 512], dt.float32,
                                          tag=f"pd{dj}", name=f"pd{dj}")
                               for dj in range(2)]
                        for g in range(cfg.NFG):
                            for dj in range(2):
                                nc.tensor.matmul(
                                    pds[dj][:], x3q[g][:, tcs],
                                    wd[g][:, 512 * dj:512 * (dj + 1)],
                                    start=(g == 0), stop=False)
                        for dj in range(2):
                            nc.tensor.matmul(
                                pds[dj][:], x3a_sb[:, tca],
                                bd_sb[:, cfg.DDQ * q + 512 * dj:
                                      cfg.DDQ * q + 512 * (dj + 1)],
                                start=False, stop=True)
                        for dj in range(2):
                            yb = yb_pool.tile([128, 512], dt.bfloat16,
                                              tag="yb")
                            nc.scalar.copy(yb[:], pds[dj][:])
                            nc.sync.dma_start(
                                rs_in[q][tca, 512 * dj:512 * (dj + 1)],
                                yb[:])
                nc.gpsimd.collective_compute(
                    "AllToAll", ALU.bypass, replica_groups=rg,
                    ins=[rs_in[q][:, :].opt()],
                    outs=[a2a_out[q][:, :].opt()],
                )
                # reduce one quarter behind: reduce(q-1)'s a2a-gated loads
                # sit after pass q's compute loads, so they cannot block them
                if q >= 1:
                    reduce_q(q - 1)
            reduce_q(cfg.n_q - 1)

    nc.compile()
    return nc


# ----------------- host side -----------------

_CACHE = {}


def _get_graph(cfg: Cfg):
    key = (cfg.D, cfg.T, cfg.F, cfg.ncores)
    if key not in _CACHE:
        _CACHE[key] = build_graph(cfg)
    return _CACHE[key]


def _prep_inputs(cfg: Cfg, inputs):
    """Shard + lay out the full inputs for each core. Marshalling only."""
    D, T, F, FP, FS, R = cfg.D, cfg.T, cfg.F, cfg.FP, cfg.FS, cfg.R
    blk = cfg.block
    dperm = _dperm(D)
    fperm = _fperm_local(cfg)

    x = inputs["x"]
    xT = np.ascontiguousarray(x.T[dperm]).astype(BF16)

    def pack_rows(packed, absmax):
        """gate/up style: packed [F*D/2] -> per-core (bytes [D/2, FS], am plane)."""
        b = (packed.astype(np.int64) & 0xFF).astype(np.uint8).reshape(F, D // 2)
        b = np.concatenate([b, np.zeros((FP - F, D // 2), np.uint8)], 0)
        am = absmax.reshape(F, D // blk).astype(np.float32)
        am = np.concatenate([am, np.zeros((FP - F, D // blk), np.float32)], 0)
        outs = []
        for i in range(cfg.ncores):
            bs = b[FS * i:FS * (i + 1)][fperm]           # [FS, D/2]
            ams = am[FS * i:FS * (i + 1)][fperm]         # [FS, D/blk]
            bT = np.ascontiguousarray(bs.T)              # [D/2, FS]
            amT = np.repeat(ams.T.astype(BF16), blk // 2, axis=0)  # [D/2, FS]
            outs.append((bT, np.ascontiguousarray(amT)))
        return outs

    def pack_down(packed, absmax):
        """down: packed [D*F/2] -> per-core (bytes [FS/2, D], am plane [FS/2, D])."""
        b = (packed.astype(np.int64) & 0xFF).astype(np.uint8).reshape(D, F // 2)
        b = np.concatenate([b, np.zeros((D, (FP - F) // 2), np.uint8)], 1)
        am = absmax.reshape(D, F // blk).astype(np.float32)
        am = np.concatenate([am, np.zeros((D, (FP - F) // blk), np.float32)], 1)
        outs = []
        npairs = FS // 2
        nblk = FS // blk
        for i in range(cfg.ncores):
            bs = b[:, npairs * i:npairs * (i + 1)]       # [D, FS/2]
            ams = am[:, nblk * i:nblk * (i + 1)]         # [D, FS/blk]
            bT = np.ascontiguousarray(bs.T)              # [FS/2, D]
            amT = np.repeat(ams.T.astype(BF16), blk // 2, axis=0)  # [FS/2, D]
            outs.append((bT, np.ascontiguousarray(amT)))
        return outs

    g = pack_rows(inputs["w_gate_packed"], inputs["w_gate_absmax"])
    u = pack_rows(inputs["w_up_packed"], inputs["w_up_absmax"])
    d = pack_down(inputs["w_down_packed"], inputs["w_down_absmax"])

    code_rep = np.broadcast_to(
        inputs["code"].astype(BF16).astype(np.float32)[None, :], (128, 16)
    ).copy()
    a_gu = np.concatenate(
        [inputs["w_gate_lora_a"], inputs["w_up_lora_a"]], axis=1
    )[dperm].astype(BF16)

    def pad_cols(m):
        return np.concatenate([m, np.zeros((m.shape[0], FP - F), m.dtype)], 1)

    b_g_full = pad_cols(inputs["w_gate_lora_b"].astype(np.float32))
    b_u_full = pad_cols(inputs["w_up_lora_b"].astype(np.float32))
    a_d_full = np.concatenate(
        [inputs["w_down_lora_a"].astype(np.float32),
         np.zeros((FP - F, R), np.float32)], 0
    )
    b_d = inputs["w_down_lora_b"].astype(BF16)

    in_maps = []
    for i in range(cfg.ncores):
        fsl = slice(FS * i, FS * (i + 1))
        in_maps.append({
            "xT": xT,
            "g_bytes": g[i][0], "g_am": g[i][1],
            "u_bytes": u[i][0], "u_am": u[i][1],
            "d_bytes": d[i][0], "d_am": d[i][1],
            "code_rep": code_rep,
            "a_gu": a_gu,
            "b_g": np.ascontiguousarray(b_g_full[:, fsl][:, fperm]).astype(BF16),
            "b_u": np.ascontiguousarray(b_u_full[:, fsl][:, fperm]).astype(BF16),
            "a_d": np.ascontiguousarray(a_d_full[fsl][fperm]).astype(BF16),
            "b_d": b_d,
        })
    return in_maps


def run(cfg: Cfg, inputs, trace=False, **kwargs):
    nc = _get_graph(cfg)
    in_maps = _prep_inputs(cfg, inputs)
    res = run_bass_kernel_spmd(
        nc, in_maps, core_ids=list(range(cfg.ncores)), trace=trace, **kwargs
    )
    y = np.concatenate(
        [np.concatenate([np.asarray(res.results[i][f"y_q{q}"])
                         for q in range(cfg.n_q)], 1)
         for i in range(cfg.ncores)], 0
    )
    return y, res


def kernel(**inputs) -> np.ndarray:
    cfg = Cfg()
    y, _ = run(cfg, inputs)
    return y.astype(np.float32)


# revision 27
# speedup vs baseline: 1.0604x; 1.0604x over previous
"""Trainium2 Bass kernel for nn_MixedGatedMLP (4-bit quantized gated MLP + LoRA).

Strategy: tensor-parallel over d_ff across 8 NeuronCores (F padded 11008->11264,
1408 rows/core).  V2 pipeline: a free-running dequant SWEEP (DVE is_equal
masked-sum + ACT u8->bf16 converts) writes bf16 weights to per-window DRAM
staging tensors; the matmul stages consume them so TensorE never shares an
engine queue with the sweep.  Stage B (gate/up GEMMs + silu-gating, gating
mult on GPSIMD) produces x3; stage C (down GEMM, one pass per 1024-wide
d-quarter) feeds a ReduceScatter that sums the 8 partial y3 and scatters
tokens, so core i directly receives its 512 output tokens.
"""

import sys

for _p in ("/opt/trn_rl_repo", "/root/.axon_site/_ro/trn_rl_repo"):
    if _p not in sys.path:
        sys.path.append(_p)

from contextlib import ExitStack

import numpy as np
import ml_dtypes

import concourse.bass as bass
import concourse.mybir as mybir
import concourse.tile as tile
from concourse import bacc
from concourse.bass_utils import run_bass_kernel_spmd

BF16 = ml_dtypes.bfloat16
NCORES = 8
ALU = mybir.AluOpType
AFT = mybir.ActivationFunctionType


class Cfg:
    def __init__(self, D=4096, T=4096, F=11008, R=16, block=64, ncores=8):
        self.D = D              # d_model
        self.T = T              # tokens
        self.F = F              # true d_ff
        self.R = R              # lora rank
        self.block = block      # absmax block size
        self.ncores = ncores
        unit = 2 * block * ncores
        self.FP = ((F + unit - 1) // unit) * unit     # 11264
        self.FS = self.FP // ncores                   # 1408 f rows/core
        self.TS = T // ncores                         # 512 out tokens/core
        self.DP = D // 256                            # 16 byte-row chunks
        self.NT = T // 512                            # 8 token tiles
        # gate/up f-windows (narrow first => short PE prologue)
        self.fwins = [(0, 256), (256, 512), (768, 640)]
        assert sum(w for _, w in self.fwins) == self.FS
        # down d-quarters
        self.DDQ = 1024
        self.n_q = D // self.DDQ                      # 4
        self.NFG = self.FS // 128                     # 11 f-groups
        # down byte-pair chunks (pairs of f): [(j0, j1), ...] <=128 each
        self.j_chunks = []
        j0 = 0
        npairs = self.FS // 2
        while j0 < npairs:
            j1 = min(j0 + 128, npairs)
            self.j_chunks.append((j0, j1))
            j0 = j1
        self.use_silu = True


def _dperm(D):
    """Row order of xT: per 256-d chunk, evens then odds."""
    idx = []
    for c in range(D // 256):
        base = 256 * c
        idx.extend(range(base, base + 256, 2))
        idx.extend(range(base + 1, base + 256, 2))
    return np.array(idx)


def _fperm_local(cfg):
    """Within-shard f order: per down j-chunk, even f (2j) then odd f (2j+1)."""
    idx = []
    for (j0, j1) in cfg.j_chunks:
        idx.extend(2 * j for j in range(j0, j1))
        idx.extend(2 * j + 1 for j in range(j0, j1))
    return np.array(idx)


def build_graph(cfg: Cfg):
    nc = bacc.Bacc(None, num_devices=cfg.ncores)
    dt = mybir.dt
    D, T, FS, R = cfg.D, cfg.T, cfg.FS, cfg.R

    # ---- external inputs (per-core) ----
    xT = nc.dram_tensor("xT", [D, T], dt.bfloat16, kind="ExternalInput")
    g_bytes = nc.dram_tensor("g_bytes", [D // 2, FS], dt.uint8, kind="ExternalInput")
    u_bytes = nc.dram_tensor("u_bytes", [D // 2, FS], dt.uint8, kind="ExternalInput")
    d_bytes = nc.dram_tensor("d_bytes", [FS // 2, D], dt.uint8, kind="ExternalInput")
    g_am = nc.dram_tensor("g_am", [D // 2, FS], dt.bfloat16, kind="ExternalInput")
    u_am = nc.dram_tensor("u_am", [D // 2, FS], dt.bfloat16, kind="ExternalInput")
    d_am = nc.dram_tensor("d_am", [FS // 2, D], dt.bfloat16, kind="ExternalInput")
    code_rep = nc.dram_tensor("code_rep", [128, 16], dt.float32, kind="ExternalInput")
    a_gu = nc.dram_tensor("a_gu", [D, 2 * R], dt.bfloat16, kind="ExternalInput")
    b_g = nc.dram_tensor("b_g", [R, FS], dt.bfloat16, kind="ExternalInput")
    b_u = nc.dram_tensor("b_u", [R, FS], dt.bfloat16, kind="ExternalInput")
    a_d = nc.dram_tensor("a_d", [FS, R], dt.bfloat16, kind="ExternalInput")
    b_d = nc.dram_tensor("b_d", [R, D], dt.bfloat16, kind="ExternalInput")

    y_q = [nc.dram_tensor(f"y_q{q}", [cfg.TS, cfg.DDQ], dt.bfloat16,
                          kind="ExternalOutput")
           for q in range(cfg.n_q)]

    # ---- internal DRAM staging ----
    wg_w = [nc.dram_tensor(f"wg_w{i}", [D, fw], dt.bfloat16, kind="Internal")
            for i, (_, fw) in enumerate(cfg.fwins)]
    wu_w = [nc.dram_tensor(f"wu_w{i}", [D, fw], dt.bfloat16, kind="Internal")
            for i, (_, fw) in enumerate(cfg.fwins)]
    wd_q = [nc.dram_tensor(f"wd_q{q}", [FS, cfg.DDQ], dt.bfloat16, kind="Internal")
            for q in range(cfg.n_q)]
    x3_dram = nc.dram_tensor("x3_dram", [FS, T], dt.bfloat16, kind="Internal")
    sg_dram = nc.dram_tensor("sg_dram", [FS, T], dt.bfloat16, kind="Internal")
    rs_in = [nc.dram_tensor(f"rs_in{q}", [T, cfg.DDQ], dt.bfloat16, kind="Internal")
             for q in range(cfg.n_q)]
    a2a_out = [nc.dram_tensor(f"a2a_out{q}", [T, cfg.DDQ], dt.bfloat16,
                              kind="Internal")
               for q in range(cfg.n_q)]

    rg = [list(range(cfg.ncores))]

    with tile.TileContext(nc) as tc, ExitStack() as ctx:
        const_pool = ctx.enter_context(tc.tile_pool(name="const", bufs=1))
        code_sb = const_pool.tile([128, 16], dt.float32)
        nc.sync.dma_start(code_sb[:], code_rep[:])
        agu_sb = const_pool.tile([128, D // 128, 2 * R], dt.bfloat16)
        nc.sync.dma_start(agu_sb[:], a_gu.rearrange("(c p) r -> p c r", p=128))
        bg_sb = const_pool.tile([R, FS], dt.bfloat16)
        nc.sync.dma_start(bg_sb[:], b_g[:])
        bu_sb = const_pool.tile([R, FS], dt.bfloat16)
        nc.sync.dma_start(bu_sb[:], b_u[:])
        ad_sb = const_pool.tile([128, FS // 128, R], dt.bfloat16)
        nc.sync.dma_start(ad_sb[:], a_d.rearrange("(c p) r -> p c r", p=128))
        bd_sb = const_pool.tile([R, D], dt.bfloat16)
        nc.sync.dma_start(bd_sb[:], b_d[:])
        # persistent lora activations (R=16 partitions)
        xag_sb = const_pool.tile([R, T], dt.bfloat16)
        xau_sb = const_pool.tile([R, T], dt.bfloat16)
        x3a_sb = const_pool.tile([R, T], dt.bfloat16)

        # ============ dequant sweep (DVE + ACT) ============
        # These pools stay open for the whole kernel: if their SBUF zones
        # were released and reused by the matmul-stage pools, the reuse
        # dependency would serialize the matmuls behind the entire sweep.
        dqs = ctx.enter_context(tc.tile_pool(name="dqs", bufs=2))
        wst = ctx.enter_context(tc.tile_pool(name="wst", bufs=2))
        if True:
            def dq_chain(B, S, fw2, extract, W):
                """Masked-sum codebook lookup on byte tile B [128, fw2]:
                extract nibble per `extract` ('hi' | 'lo' | 'stacked' using
                the per-partition shift vector), then W = code[n] * S."""
                U = dqs.tile([128, fw2], dt.uint8, tag="u")
                if extract == "hi":
                    nc.vector.tensor_scalar(U[:], B[:], 4, None,
                                            ALU.logical_shift_right)
                elif extract == "lo":
                    nc.vector.tensor_scalar(U[:], B[:], 15, None,
                                            ALU.bitwise_and)
                elif extract == "hilo":  # U = [B>>4 | B&15], B [128, fw2/2]
                    h = fw2 // 2
                    nc.vector.tensor_scalar(U[:, 0:h], B[:], 4, None,
                                            ALU.logical_shift_right)
                    nc.vector.tensor_scalar(U[:, h:], B[:], 15, None,
                                            ALU.bitwise_and)
                else:  # stacked: hi on partitions [0,64), lo on [64,128)
                    nc.vector.tensor_scalar(U[0:64, :], B[0:64, :], 4, None,
                                            ALU.logical_shift_right)
                    nc.vector.tensor_scalar(U[64:128, :], B[64:128, :], 15,
                                            None, ALU.bitwise_and)
                X = dqs.tile([128, fw2], dt.bfloat16, tag="x")
                nc.scalar.copy(X[:], U[:])
                acc0 = dqs.tile([128, fw2], dt.bfloat16, tag="a0")
                acc1 = dqs.tile([128, fw2], dt.bfloat16, tag="a1")
                tk = dqs.tile([128, fw2], dt.bfloat16, tag="tk")
                nc.vector.tensor_scalar(acc0[:], X[:], 0.0, code_sb[:, 0:1],
                                        ALU.is_equal, ALU.mult)
                nc.vector.tensor_scalar(acc1[:], X[:], 1.0, code_sb[:, 1:2],
                                        ALU.is_equal, ALU.mult)
                for k in range(2, 16):
                    acc = acc0 if (k % 2 == 0) else acc1
                    nc.vector.tensor_scalar(tk[:], X[:], float(k),
                                            code_sb[:, k:k + 1],
                                            ALU.is_equal, ALU.mult)
                    nc.vector.tensor_tensor(acc[:], acc[:], tk[:], ALU.add)
                nc.vector.tensor_tensor(acc0[:], acc0[:], acc1[:], ALU.add)
                nc.vector.tensor_tensor(W[:], acc0[:], S[:], ALU.mult)

            # gate/up. Window 0 keeps [g|u]-paired chains; windows 1+ are
            # swept gate-matrix-first then up ([hi|lo] pairing keeps the same
            # chain width), so stage B's gate GEMMs unlock half a window
            # earlier and fill the window-boundary PE valleys.
            for wi, (f0, fw) in enumerate(cfg.fwins):
                if wi == 0:
                    for c in range(cfg.DP):
                        rsl = slice(128 * c, 128 * (c + 1))
                        B2 = dqs.tile([128, 2 * fw], dt.uint8, tag="bq")
                        S2 = dqs.tile([128, 2 * fw], dt.bfloat16, tag="sq")
                        nc.sync.dma_start(B2[:, 0:fw], g_bytes[rsl, f0:f0 + fw])
                        nc.sync.dma_start(B2[:, fw:], u_bytes[rsl, f0:f0 + fw])
                        nc.sync.dma_start(S2[:, 0:fw], g_am[rsl, f0:f0 + fw])
                        nc.sync.dma_start(S2[:, fw:], u_am[rsl, f0:f0 + fw])
                        for ni, ex in ((0, "hi"), (1, "lo")):
                            W2 = wst.tile([128, 2 * fw], dt.bfloat16,
                                          tag="wsh")
                            dq_chain(B2, S2, 2 * fw, ex, W2)
                            osl = slice(256 * c + 128 * ni,
                                        256 * c + 128 * (ni + 1))
                            nc.sync.dma_start(wg_w[wi][osl, :], W2[:, 0:fw])
                            nc.sync.dma_start(wu_w[wi][osl, :], W2[:, fw:])
                    continue
                for bsrc, asrc, wdst in ((g_bytes, g_am, wg_w[wi]),
                                         (u_bytes, u_am, wu_w[wi])):
                    for c in range(cfg.DP):
                        rsl = slice(128 * c, 128 * (c + 1))
                        B1 = dqs.tile([128, fw], dt.uint8, tag="b1")
                        S2 = dqs.tile([128, 2 * fw], dt.bfloat16, tag="sq")
                        nc.sync.dma_start(B1[:], bsrc[rsl, f0:f0 + fw])
                        nc.sync.dma_start(S2[:, 0:fw], asrc[rsl, f0:f0 + fw])
                        nc.sync.dma_start(S2[:, fw:], asrc[rsl, f0:f0 + fw])
                        W2 = wst.tile([128, 2 * fw], dt.bfloat16, tag="wsh")
                        dq_chain(B1, S2, 2 * fw, "hilo", W2)
                        nc.sync.dma_start(
                            wdst[256 * c:256 * c + 128, :], W2[:, 0:fw])
                        nc.sync.dma_start(
                            wdst[256 * c + 128:256 * c + 256, :], W2[:, fw:])
            # down, quarter-major
            for q in range(cfg.n_q):
                dd = slice(cfg.DDQ * q, cfg.DDQ * (q + 1))
                for ic, (j0, j1) in enumerate(cfg.j_chunks):
                    pc = j1 - j0
                    B2 = dqs.tile([128, cfg.DDQ], dt.uint8, tag="bq")
                    S2 = dqs.tile([128, cfg.DDQ], dt.bfloat16, tag="sq")
                    if pc == 128:
                        nc.sync.dma_start(B2[:], d_bytes[j0:j1, dd])
                        nc.sync.dma_start(S2[:], d_am[j0:j1, dd])
                        for ni, ex in ((0, "hi"), (1, "lo")):
                            W2 = wst.tile([128, cfg.DDQ], dt.bfloat16,
                                          tag="wsh")
                            dq_chain(B2, S2, cfg.DDQ, ex, W2)
                            osl = slice(256 * ic + 128 * ni,
                                        256 * ic + 128 * (ni + 1))
                            nc.sync.dma_start(wd_q[q][osl, :], W2[:])
                    else:
                        # ragged tail (64 pairs): duplicate the byte rows on
                        # both partition halves, extract hi on [0,64) and lo
                        # on [64,128) via the per-partition shift vector
                        nc.sync.dma_start(B2[0:pc, :], d_bytes[j0:j1, dd])
                        nc.sync.dma_start(B2[pc:2 * pc, :], d_bytes[j0:j1, dd])
                        nc.sync.dma_start(S2[0:pc, :], d_am[j0:j1, dd])
                        nc.sync.dma_start(S2[pc:2 * pc, :], d_am[j0:j1, dd])
                        W2 = wst.tile([128, cfg.DDQ], dt.bfloat16, tag="wsh")
                        dq_chain(B2, S2, cfg.DDQ, "stacked", W2)
                        nc.sync.dma_start(
                            wd_q[q][256 * ic:256 * ic + 2 * pc, :],
                            W2[0:2 * pc, :])

        # ================= stage B: gate/up matmuls -> x3 =================
        with (
            tc.tile_pool(name="w", bufs=34) as w_pool,
            tc.tile_pool(name="xt", bufs=34) as xt_pool,
            tc.tile_pool(name="p1", bufs=3) as p1_pool,
            tc.tile_pool(name="ps1", bufs=2, space="PSUM") as psum1,
            tc.tile_pool(name="psa", bufs=2, space="PSUM") as psuma,
        ):
            # lora prologue: x@[Ag|Au] for all token tiles (only needs x)
            for t in range(cfg.NT):
                tt = slice(512 * t, 512 * (t + 1))
                xts = []
                for ci in range(2 * cfg.DP):
                    xt_t = xt_pool.tile([128, 512], dt.bfloat16, tag="xt")
                    nc.sync.dma_start(xt_t[:], xT[128 * ci:128 * (ci + 1), tt])
                    xts.append(xt_t)
                for ri, dst in ((0, xag_sb), (1, xau_sb)):
                    pa2 = psuma.tile([R, 512], dt.float32, tag="pa2")
                    for ci in range(2 * cfg.DP):
                        nc.tensor.matmul(pa2[:],
                                         agu_sb[:, ci, R * ri:R * (ri + 1)],
                                         xts[ci][:], start=(ci == 0),
                                         stop=(ci == 2 * cfg.DP - 1))
                    nc.scalar.copy(dst[:, tt], pa2[:])

            # pass list mirrors the sweep order: window 0 coupled, then
            # gate-only / up-only passes per remaining window.  Gate passes
            # stash silu(y1) to DRAM; up passes reload it and finish x3.
            passes = [("gu", 0)]
            for wi in range(1, len(cfg.fwins)):
                passes += [("g", wi), ("u", wi)]
            for kind, wi in passes:
                f0, fw = cfg.fwins[wi]
                do_g = kind in ("gu", "g")
                do_u = kind in ("gu", "u")
                wg = []
                wu = []
                for ci in range(2 * cfg.DP):
                    if do_g:
                        tg_ = w_pool.tile([128, fw], dt.bfloat16, tag="wg")
                        nc.sync.dma_start(
                            tg_[:], wg_w[wi][128 * ci:128 * (ci + 1), :])
                        wg.append(tg_)
                    if do_u:
                        tu_ = w_pool.tile([128, fw], dt.bfloat16, tag="wu")
                        nc.sync.dma_start(
                            tu_[:], wu_w[wi][128 * ci:128 * (ci + 1), :])
                        wu.append(tu_)
                for t in range(cfg.NT):
                    tt = slice(512 * t, 512 * (t + 1))
                    xts = []
                    for ci in range(2 * cfg.DP):
                        xt_t = xt_pool.tile([128, 512], dt.bfloat16, tag="xt")
                        nc.sync.dma_start(
                            xt_t[:], xT[128 * ci:128 * (ci + 1), tt])
                        xts.append(xt_t)
                    for g in range(fw // 128):
                        fg = slice(128 * g, 128 * (g + 1))
                        fga = slice(f0 + 128 * g, f0 + 128 * (g + 1))
                        sg = None
                        if do_g:
                            pg = psum1.tile([128, 512], dt.float32, tag="pg")
                            for ci in range(2 * cfg.DP):
                                nc.tensor.matmul(pg[:], wg[ci][:, fg],
                                                 xts[ci][:],
                                                 start=(ci == 0), stop=False)
                            nc.tensor.matmul(pg[:], bg_sb[:, fga],
                                             xag_sb[:, tt],
                                             start=False, stop=True)
                            # silu on ACT (DVE is busy with the sweep)
                            sg = p1_pool.tile([128, 512], dt.bfloat16,
                                              tag="sg")
                            nc.scalar.activation(sg[:], pg[:], AFT.Silu)
                            if kind == "g":
                                nc.sync.dma_start(sg_dram[fga, tt], sg[:])
                        if do_u:
                            pu = psum1.tile([128, 512], dt.float32, tag="pu")
                            for ci in range(2 * cfg.DP):
                                nc.tensor.matmul(pu[:], wu[ci][:, fg],
                                                 xts[ci][:],
                                                 start=(ci == 0), stop=False)
                            nc.tensor.matmul(pu[:], bu_sb[:, fga],
                                             xau_sb[:, tt],
                                             start=False, stop=True)
                            # gating mult on GPSIMD (no PSUM access, so ACT
                            # stages pu into SBUF)
                            pu_s = p1_pool.tile([128, 512], dt.bfloat16,
                                                tag="pus")
                            nc.scalar.copy(pu_s[:], pu[:])
                            if kind == "u":
                                sg = p1_pool.tile([128, 512], dt.bfloat16,
                                                  tag="sgl")
                                nc.sync.dma_start(sg[:], sg_dram[fga, tt])
                            x3t = p1_pool.tile([128, 512], dt.bfloat16,
                                               tag="x3t")
                            nc.gpsimd.tensor_tensor(x3t[:], sg[:], pu_s[:],
                                                    ALU.mult)
                            nc.sync.dma_start(x3_dram[fga, tt], x3t[:])

        # ================= stage C: down matmuls + AllToAll+reduce ========
        with (
            tc.tile_pool(name="wd", bufs=24) as wd_pool,
            tc.tile_pool(name="x3q", bufs=24) as x3q_pool,
            tc.tile_pool(name="yb", bufs=6) as yb_pool,
            tc.tile_pool(name="red", bufs=cfg.ncores + 2) as red_pool,
            tc.tile_pool(name="ps2", bufs=3, space="PSUM") as psum2,
            tc.tile_pool(name="psb", bufs=2, space="PSUM") as psumb,
        ):
            def reduce_q(q):
                """Sum the 8 exchanged partials for this core's tokens (DVE;
                idle once the sweep is done)."""
                for ts in range(cfg.TS // 128):
                    for dj in range(2):
                        dsl = slice(512 * dj, 512 * (dj + 1))
                        parts = []
                        for j in range(cfg.ncores):
                            pt = red_pool.tile([128, 512], dt.bfloat16,
                                               tag="rp")
                            r0 = cfg.TS * j + 128 * ts
                            nc.sync.dma_start(
                                pt[:], a2a_out[q][r0:r0 + 128, dsl])
                            parts.append(pt)
                        for lvl in (4, 2):
                            for j in range(lvl):
                                nc.vector.tensor_tensor(
                                    parts[j][:], parts[j][:],
                                    parts[j + lvl][:], ALU.add)
                        yf = red_pool.tile([128, 512], dt.bfloat16, tag="yf")
                        nc.vector.tensor_tensor(yf[:], parts[0][:],
                                                parts[1][:], ALU.add)
                        nc.sync.dma_start(
                            y_q[q][128 * ts:128 * (ts + 1), dsl], yf[:])

            for q in range(cfg.n_q):
                dds = slice(cfg.DDQ * q, cfg.DDQ * (q + 1))
                wd = []
                for g in range(cfg.NFG):
                    wt = wd_pool.tile([128, cfg.DDQ], dt.bfloat16, tag="wd")
                    nc.sync.dma_start(
                        wt[:], wd_q[q][128 * g:128 * (g + 1), :])
                    wd.append(wt)
                for tq in range(cfg.NT):  # 512-token quads
                    tqs = slice(512 * tq, 512 * (tq + 1))
                    x3q = []
                    for g in range(cfg.NFG):
                        xq = x3q_pool.tile([128, 512], dt.bfloat16, tag="x3q")
                        nc.sync.dma_start(
                            xq[:], x3_dram[128 * g:128 * (g + 1), tqs])
                        x3q.append(xq)
                    if q == 0:
                        # x3^T @ A_d for these tokens (needed by all lora
                        # stops); do all 4 subtiles first so the ACT copies
                        # pipeline ahead of the stop matmuls
                        for ts in range(4):
                            tcs = slice(128 * ts, 128 * (ts + 1))
                            tca = slice(512 * tq + 128 * ts,
                                        512 * tq + 128 * (ts + 1))
                            pa = psumb.tile([R, 128], dt.float32, tag="pa")
                            for g in range(cfg.NFG):
                                nc.tensor.matmul(pa[:], ad_sb[:, g, :],
                                                 x3q[g][:, tcs],
                                                 start=(g == 0),
                                                 stop=(g == cfg.NFG - 1))
                            nc.scalar.copy(x3a_sb[:, tca], pa[:])
                    for ts in range(4):
                        tcs = slice(128 * ts, 128 * (ts + 1))
                        tca = slice(512 * tq + 128 * ts,
                                    512 * tq + 128 * (ts + 1))
                        pds = [psum2.tile([128, 512], dt.float32,
                                          tag=f"pd{dj}", name=f"pd{dj}")
                               for dj in range(2)]
                        for g in range(cfg.NFG):
                            for dj in range(2):
                                nc.tensor.matmul(
                                    pds[dj][:], x3q[g][:, tcs],
                                    wd[g][:, 512 * dj:512 * (dj + 1)],
                                    start=(g == 0), stop=False)
                        for dj in range(2):
                            nc.tensor.matmul(
                                pds[dj][:], x3a_sb[:, tca],
                                bd_sb[:, cfg.DDQ * q + 512 * dj:
                                      cfg.DDQ * q + 512 * (dj + 1)],
                                start=False, stop=True)
                        for dj in range(2):
                            yb = yb_pool.tile([128, 512], dt.bfloat16,
                                              tag="yb")
                            nc.scalar.copy(yb[:], pds[dj][:])
                            nc.sync.dma_start(
                                rs_in[q][tca, 512 * dj:512 * (dj + 1)],
                                yb[:])
                nc.gpsimd.collective_compute(
                    "AllToAll", ALU.bypass, replica_groups=rg,
                    ins=[rs_in[q][:, :].opt()],
                    outs=[a2a_out[q][:, :].opt()],
                )
                # reduce one quarter behind: reduce(q-1)'s a2a-gated loads
                # sit after pass q's compute loads, so they cannot block them
                if q >= 1:
                    reduce_q(q - 1)
            reduce_q(cfg.n_q - 1)

    nc.compile()
    return nc


# ----------------- host side -----------------

_CACHE = {}


def _get_graph(cfg: Cfg):
    key = (cfg.D, cfg.T, cfg.F, cfg.ncores)
    if key not in _CACHE:
        _CACHE[key] = build_graph(cfg)
    return _CACHE[key]


def _prep_inputs(cfg: Cfg, inputs):
    """Shard + lay out the full inputs for each core. Marshalling only."""
    D, T, F, FP, FS, R = cfg.D, cfg.T, cfg.F, cfg.FP, cfg.FS, cfg.R
    blk = cfg.block
    dperm = _dperm(D)
    fperm = _fperm_local(cfg)

    x = inputs["x"]
    xT = np.ascontiguousarray(x.T[dperm]).astype(BF16)

    def pack_rows(packed, absmax):
        """gate/up style: packed [F*D/2] -> per-core (bytes [D/2, FS], am plane)."""
        b = (packed.astype(np.int64) & 0xFF).astype(np.uint8).reshape(F, D // 2)
        b = np.concatenate([b, np.zeros((FP - F, D // 2), np.uint8)], 0)
        am = absmax.reshape(F, D // blk).astype(np.float32)
        am = np.concatenate([am, np.zeros((FP - F, D // blk), np.float32)], 0)
        outs = []
        for i in range(cfg.ncores):
            bs = b[FS * i:FS * (i + 1)][fperm]           # [FS, D/2]
            ams = am[FS * i:FS * (i + 1)][fperm]         # [FS, D/blk]
            bT = np.ascontiguousarray(bs.T)              # [D/2, FS]
            amT = np.repeat(ams.T.astype(BF16), blk // 2, axis=0)  # [D/2, FS]
            outs.append((bT, np.ascontiguousarray(amT)))
        return outs

    def pack_down(packed, absmax):
        """down: packed [D*F/2] -> per-core (bytes [FS/2, D], am plane [FS/2, D])."""
        b = (packed.astype(np.int64) & 0xFF).astype(np.uint8).reshape(D, F // 2)
        b = np.concatenate([b, np.zeros((D, (FP - F) // 2), np.uint8)], 1)
        am = absmax.reshape(D, F // blk).astype(np.float32)
        am = np.concatenate([am, np.zeros((D, (FP - F) // blk), np.float32)], 1)
        outs = []
        npairs = FS // 2
        nblk = FS // blk
        for i in range(cfg.ncores):
            bs = b[:, npairs * i:npairs * (i + 1)]       # [D, FS/2]
            ams = am[:, nblk * i:nblk * (i + 1)]         # [D, FS/blk]
            bT = np.ascontiguousarray(bs.T)              # [FS/2, D]
            amT = np.repeat(ams.T.astype(BF16), blk // 2, axis=0)  # [FS/2, D]
            outs.append((bT, np.ascontiguousarray(amT)))
        return outs

    g = pack_rows(inputs["w_gate_packed"], inputs["w_gate_absmax"])
    u = pack_rows(inputs["w_up_packed"], inputs["w_up_absmax"])
    d = pack_down(inputs["w_down_packed"], inputs["w_down_absmax"])

    code_rep = np.broadcast_to(
        inputs["code"].astype(BF16).astype(np.float32)[None, :], (128, 16)
    ).copy()
    a_gu = np.concatenate(
        [inputs["w_gate_lora_a"], inputs["w_up_lora_a"]], axis=1
    )[dperm].astype(BF16)

    def pad_cols(m):
        return np.concatenate([m, np.zeros((m.shape[0], FP - F), m.dtype)], 1)

    b_g_full = pad_cols(inputs["w_gate_lora_b"].astype(np.float32))
    b_u_full = pad_cols(inputs["w_up_lora_b"].astype(np.float32))
    a_d_full = np.concatenate(
        [inputs["w_down_lora_a"].astype(np.float32),
         np.zeros((FP - F, R), np.float32)], 0
    )
    b_d = inputs["w_down_lora_b"].astype(BF16)

    in_maps = []
    for i in range(cfg.ncores):
        fsl = slice(FS * i, FS * (i + 1))
        in_maps.append({
            "xT": xT,
            "g_bytes": g[i][0], "g_am": g[i][1],
            "u_bytes": u[i][0], "u_am": u[i][1],
            "d_bytes": d[i][0], "d_am": d[i][1],
            "code_rep": code_rep,
            "a_gu": a_gu,
            "b_g": np.ascontiguousarray(b_g_full[:, fsl][:, fperm]).astype(BF16),
            "b_u": np.ascontiguousarray(b_u_full[:, fsl][:, fperm]).astype(BF16),
            "a_d": np.ascontiguousarray(a_d_full[fsl][fperm]).astype(BF16),
            "b_d": b_d,
        })
    return in_maps


def run(cfg: Cfg, inputs, trace=False, **kwargs):
    nc = _get_graph(cfg)
    in_maps = _prep_inputs(cfg, inputs)
    res = run_bass_kernel_spmd(
        nc, in_maps, core_ids=list(range(cfg.ncores)), trace=trace, **kwargs
    )
    y = np.concatenate(
        [np.concatenate([np.asarray(res.results[i][f"y_q{q}"])
                         for q in range(cfg.n_q)], 1)
         for i in range(cfg.ncores)], 0
    )
    return y, res


def kernel(**inputs) -> np.ndarray:
    cfg = Cfg()
    y, _ = run(cfg, inputs)
    return y.astype(np.float32)
